# revision 23
# baseline (speedup 1.0000x reference)
"""Causal single-head attention (B=4, S=2048, D=1024, fp32) on 8 TRN2 NeuronCores.

Sharding: core c <-> (batch c//2, parity c%2). Each core owns 8 of the 16
128-row query tiles of its batch, chosen so total causal work is balanced
(parity 0: tiles {0,3,4,7,8,11,12,15}, parity 1: {1,2,5,6,9,10,13,14}; both
sum to 68 causal blocks). All compute is bf16 into fp32 PSUM (tolerance 2e-2
admits it; bf16 runs the PE at full rate and halves SBUF/DMA).

Attention is computed transposed: scoresT[k,q] = sum_e kt[e].T @ qt[e], so the
exp output P^T[k,q] lands in SBUF directly in the layout the PV matmul needs
as its stationary operand — no PE transposes, no PSUM->SBUF P copies, no
cross-engine round-trips on the PE critical path. Softmax denominators come
from a 1-column matmul against a ones vector reusing the already-loaded P^T
stationary. Causal masking is data-driven (SPMD-uniform program): per-parity
mask tensors cover the last two k-tiles of each query tile's range.

Self-contained: hardcodes shapes; reads nothing from disk.
"""
import sys

import numpy as np

try:
    from concourse import bass, bacc, tile
except ImportError:  # concourse ships with the container, not this file
    for _p in ("/opt/trn_rl_repo", "/root/.axon_site/_ro/trn_rl_repo"):
        if _p not in sys.path:
            sys.path.append(_p)
    from concourse import bass, bacc, tile
from concourse import mybir
from concourse.bass_utils import run_bass_kernel_spmd

dt = mybir.dt
AF = mybir.ActivationFunctionType

B, S, D = 4, 2048, 1024
P = 128
ND = D // P          # 8 d-tiles (contraction of projections)
NE = D // P          # 8 e-tiles (output feature tiles)
NT = S // P          # 16 s-tiles per batch
SLOTS = 8            # q-tiles per core
NG = 4               # query-tile groups of 2 per core
NCORES = 8
SCALE = 1.0 / float(np.sqrt(D))
NEG = -1.0e30

# Balanced query-tile assignment per parity (both sum to 68 causal blocks),
# listed in ascending causal-extent order so slot p's extent <= ES[p].
QTILES = [
    [0, 3, 4, 7, 8, 11, 12, 15],
    [1, 2, 5, 6, 9, 10, 13, 14],
]
# Uniform processed extent (k-tiles) per slot, identical for both parities.
ES = [2 * p + 2 for p in range(SLOTS)]


def _emit_body(nc, tc, pools, aps, variant="full"):
    do_proj = variant in ("full", "proj")
    do_attn = variant in ("full", "attn", "scores")
    (sb_x, sb_w, sb_qt, sb_kt, sb_vt, sb_msk, sb_one,
     sb_rec, sb_obuf, po, ps_dn) = pools
    xT, xqT, Wq, Wk, Wv, maskA, ones, O = aps

    ones_sb = sb_one.tile([P, 1], dt.bfloat16, tag="ones", name="ones_sb")
    nc.sync.dma_start(ones_sb[:], ones[:])
    mA = sb_msk.tile([P, NT * P], dt.float32, tag="mA", name="mA")
    nc.sync.dma_start(mA[:], maskA[:])

    # ---- input DMAs: x^T (all keys), xq^T (own query cols), weights (natural rows)
    xt = [sb_x.tile([P, S], dt.bfloat16, tag=f"xt{d}", name=f"xt{d}")
          for d in range(ND)]
    xqt = [sb_x.tile([P, SLOTS * P], dt.bfloat16, tag=f"xqt{d}", name=f"xqt{d}")
           for d in range(ND)]
    wk = [sb_w.tile([P, D], dt.bfloat16, tag=f"wk{d}", name=f"wk{d}") for d in range(ND)]
    wq = [sb_w.tile([P, D], dt.bfloat16, tag=f"wq{d}", name=f"wq{d}") for d in range(ND)]
    wv = [sb_w.tile([P, D], dt.bfloat16, tag=f"wv{d}", name=f"wv{d}") for d in range(ND)]
    for d in range(ND):
        nc.sync.dma_start(xt[d][:], xT[d * P:(d + 1) * P, :])
        nc.sync.dma_start(wk[d][:], Wk[d * P:(d + 1) * P, :])
    for d in range(ND):
        nc.sync.dma_start(xqt[d][:], xqT[d * P:(d + 1) * P, :])
        nc.sync.dma_start(wq[d][:], Wq[d * P:(d + 1) * P, :])
        nc.sync.dma_start(wv[d][:], Wv[d * P:(d + 1) * P, :])

    # ---- K^T projection: kt[e][:, k] = sum_d Wk[d,e]^T x^T[d,k]
    kt = [sb_kt.tile([P, S], dt.bfloat16, tag=f"kt{e}", name=f"kt{e}")
          for e in range(NE)]
    for e in range(NE) if do_proj else []:
        for h in range(2):
            kp = po.tile([P, 2 * 512], dt.float32, tag="o", name="kp")
            for d in range(ND):
                for c in range(2):
                    nc.tensor.matmul(kp[:, c * 512:(c + 1) * 512],
                                     wk[d][:, e * P:(e + 1) * P],
                                     xt[d][:, h * 1024 + c * 512:h * 1024 + (c + 1) * 512],
                                     start=(d == 0), stop=(d == ND - 1))
            nc.vector.tensor_copy(kt[e][:, h * 1024:h * 1024 + 512], kp[:, 0:512])
            nc.vector.tensor_copy(kt[e][:, h * 1024 + 512:(h + 1) * 1024], kp[:, 512:1024])

    # ---- Q^T projection (own query columns only)
    qt = [sb_qt.tile([P, SLOTS * P], dt.bfloat16, tag=f"qt{e}", name=f"qt{e}")
          for e in range(NE)]
    for e in range(NE) if do_proj else []:
        qp = po.tile([P, 2 * 512], dt.float32, tag="o", name="qp")
        for d in range(ND):
            for c in range(2):
                nc.tensor.matmul(qp[:, c * 512:(c + 1) * 512],
                                 wq[d][:, e * P:(e + 1) * P],
                                 xqt[d][:, c * 512:(c + 1) * 512],
                                 start=(d == 0), stop=(d == ND - 1))
        nc.vector.tensor_copy(qt[e][:, 0:512], qp[:, 0:512])
        nc.vector.tensor_copy(qt[e][:, 512:1024], qp[:, 512:1024])

    # ---- V projection (natural [k, e] layout): vt[t][:, e] = sum_d x^T[d,kt]^T Wv[d,e]
    vt = [sb_vt.tile([P, D], dt.bfloat16, tag=f"vt{t}", name=f"vt{t}")
          for t in range(NT)]
    for t in range(NT) if do_proj else []:
        vp = po.tile([P, 2 * 512], dt.float32, tag="o", name="vp")
        for d in range(ND):
            for c in range(2):
                nc.tensor.matmul(vp[:, c * 512:(c + 1) * 512],
                                 xt[d][:, t * P:(t + 1) * P],
                                 wv[d][:, c * 512:(c + 1) * 512],
                                 start=(d == 0), stop=(d == ND - 1))
        nc.vector.tensor_copy(vt[t][:, 0:512], vp[:, 0:512])
        nc.vector.tensor_copy(vt[t][:, 512:1024], vp[:, 512:1024])

    if not do_proj and do_attn:
        # experiment variant: load "projected" tensors directly (garbage values)
        for e in range(NE):
            nc.sync.dma_start(kt[e][:], xT[e * P:(e + 1) * P, :])
            nc.sync.dma_start(qt[e][:], xqT[e * P:(e + 1) * P, :])
        for t in range(NT):
            nc.sync.dma_start(vt[t][:], Wv[(t % ND) * P:(t % ND + 1) * P, :])

    # ---- S phase: scoresT for all own slots at once, ragged causal widths.
    # At k-tile t, slots t//2..7 are still active; slot t//2 is in its masking
    # window, so the mask always lands on columns 0:128 of the score tile.
    # exp output P^T goes to SBUF, aliased into the spent xt/xqt buffers.
    pts = [None] * NT
    if do_attn:
        for t in range(NT):
            off = (t // 2) * P
            wide = SLOTS * P - off
            s_t = po.tile([P, wide], dt.float32, tag="o", name="s_t")
            chunks = [(0, 512), (512, wide - 512)] if wide > 512 else [(0, wide)]
            for e in range(NE):
                for c0, cw in chunks:
                    nc.tensor.matmul(s_t[:, c0:c0 + cw],
                                     kt[e][:, t * P:(t + 1) * P],
                                     qt[e][:, off + c0:off + c0 + cw],
                                     start=(e == 0), stop=(e == NE - 1))
            nc.vector.tensor_add(s_t[:, 0:P], s_t[:, 0:P],
                                 mA[:, t * P:(t + 1) * P])
            xtag = f"xt{t}" if t < ND else f"xqt{t - ND}"
            pt = sb_x.tile([P, wide], dt.bfloat16, tag=xtag, name=f"pt{t}")
            nc.scalar.activation(pt[:], s_t[:], AF.Exp, scale=SCALE)
            pts[t] = pt

    # ---- PV passes: slot pairs (2pp, 2pp+1), accumulators live in PSUM
    for pp in range(NG) if (do_attn and variant != "scores") else []:
        pa, pb = 2 * pp, 2 * pp + 1
        ea, eb = ES[pa], ES[pb]
        oa = po.tile([P, D], dt.float32, tag="o", name="oa")
        ob = po.tile([P, D], dt.float32, tag="o", name="ob")
        # separate tiles: a start=True matmul clears has_written for its whole
        # PSUM bank, so the two running denominators must not share a bank
        dna = ps_dn.tile([P, 1], dt.float32, tag="dn", name="dna")
        dnb = ps_dn.tile([P, 1], dt.float32, tag="dn", name="dnb")
        for t in range(eb):
            off = (t // 2) * P
            ca = pa * P - off
            cb = pb * P - off
            pt = pts[t]
            if t < ea:
                for c in range(2):
                    nc.tensor.matmul(oa[:, c * 512:(c + 1) * 512], pt[:, ca:ca + P],
                                     vt[t][:, c * 512:(c + 1) * 512],
                                     start=(t == 0), stop=(t == ea - 1))
                nc.tensor.matmul(dna[:], pt[:, ca:ca + P], ones_sb[:],
                                 start=(t == 0), stop=(t == ea - 1))
            for c in range(2):
                nc.tensor.matmul(ob[:, c * 512:(c + 1) * 512], pt[:, cb:cb + P],
                                 vt[t][:, c * 512:(c + 1) * 512],
                                 start=(t == 0), stop=(t == eb - 1))
            nc.tensor.matmul(dnb[:], pt[:, cb:cb + P], ones_sb[:],
                             start=(t == 0), stop=(t == eb - 1))

        rec = sb_rec.tile([P, 2], dt.float32, tag="rec", name="rec")
        nc.vector.reciprocal(rec[:, 0:1], dna[:])
        nc.vector.reciprocal(rec[:, 1:2], dnb[:])
        obufa = sb_obuf.tile([P, D], dt.bfloat16, tag="obuf", name="obufa")
        obufb = sb_obuf.tile([P, D], dt.bfloat16, tag="obuf", name="obufb")
        for c in range(2):
            nc.scalar.mul(obufa[:, c * 512:(c + 1) * 512],
                          oa[:, c * 512:(c + 1) * 512], rec[:, 0:1])
            nc.scalar.mul(obufb[:, c * 512:(c + 1) * 512],
                          ob[:, c * 512:(c + 1) * 512], rec[:, 1:2])
        nc.sync.dma_start(O[pa * P:(pa + 1) * P, :], obufa[:])
        nc.sync.dma_start(O[pb * P:(pb + 1) * P, :], obufb[:])


def dedupe_ldweights(nc):
    """Drop InstLdweights whose stationary operand is already loaded.

    The PE keeps the stationary operand in the array across matmuls; Tile emits
    an Ldweights before every matmul regardless, and those loads serialize with
    the matmul stream (~92ns each). Scan each block in final (post-scheduling)
    order and delete loads identical to the previous one, tracking only PE
    instructions; reset on anything else PE-adjacent (drains, calls, branches).
    Only loads with no semaphore waits/updates are removed.
    """
    dropped = 0
    for f in nc.m.functions:
        for blk in f.blocks:
            keep = []
            last = None
            for inst in blk.instructions:
                if isinstance(inst, mybir.InstLdweights):
                    w = inst.ins[0]
                    key = (w.memref, w.offset, str(w.ap), w.dtype,
                           inst.is_transpose, inst.perf_mode)
                    si = inst.sync_info
                    clean = si is None or (not len(si.on_wait) and not len(si.on_update))
                    if key == last and clean:
                        dropped += 1
                        continue
                    last = key
                elif isinstance(inst, mybir.InstMatmult):
                    pass  # matmuls don't change the loaded stationary
                elif isinstance(inst, (mybir.InstTensorCopy, mybir.InstTensorTensor,
                                       mybir.InstActivation, mybir.InstReciprocal,
                                       mybir.InstDMACopy, mybir.InstMemset,
                                       mybir.InstEventSemaphore)):
                    pass  # other engines: PE weight state unaffected
                else:
                    last = None  # drains/calls/branches: be conservative
                keep.append(inst)
            if len(keep) != len(blk.instructions):
                del blk.instructions[:]
                for inst in keep:
                    blk.instructions.append(inst)
    return dropped


def build_program(reps: int = 1, variant: str = "full"):
    nc = bacc.Bacc("TRN2", target_bir_lowering=False, debug=False, num_devices=NCORES)

    xT_t = nc.dram_tensor("xT", [D, S], dt.bfloat16, kind="ExternalInput")
    xqT_t = nc.dram_tensor("xqT", [D, SLOTS * P], dt.bfloat16, kind="ExternalInput")
    Wq_t = nc.dram_tensor("Wq", [D, D], dt.bfloat16, kind="ExternalInput")
    Wk_t = nc.dram_tensor("Wk", [D, D], dt.bfloat16, kind="ExternalInput")
    Wv_t = nc.dram_tensor("Wv", [D, D], dt.bfloat16, kind="ExternalInput")
    maskA_t = nc.dram_tensor("maskA", [P, NT * P], dt.float32, kind="ExternalInput")
    ones_t = nc.dram_tensor("ones", [P, 1], dt.bfloat16, kind="ExternalInput")
    O_t = nc.dram_tensor("O", [SLOTS * P, D], dt.bfloat16, kind="ExternalOutput")

    aps = (xT_t.ap(), xqT_t.ap(), Wq_t.ap(), Wk_t.ap(), Wv_t.ap(),
           maskA_t.ap(), ones_t.ap(), O_t.ap())

    with tile.TileContext(nc) as tc:
        with (
            tc.tile_pool(name="x", bufs=1) as sb_x,
            tc.tile_pool(name="w", bufs=1) as sb_w,
            tc.tile_pool(name="qt", bufs=1) as sb_qt,
            tc.tile_pool(name="kt", bufs=1) as sb_kt,
            tc.tile_pool(name="vt", bufs=1) as sb_vt,
            tc.tile_pool(name="msk", bufs=1) as sb_msk,
            tc.tile_pool(name="one", bufs=1) as sb_one,
            tc.tile_pool(name="rec", bufs=2) as sb_rec,
            tc.tile_pool(name="obuf", bufs=4) as sb_obuf,
            tc.tile_pool(name="po", bufs=3, space=bass.MemorySpace.PSUM) as po,
            tc.tile_pool(name="ps_dn", bufs=2, space=bass.MemorySpace.PSUM) as ps_dn,
        ):
            pools = (sb_x, sb_w, sb_qt, sb_kt, sb_vt, sb_msk, sb_one,
                     sb_rec, sb_obuf, po, ps_dn)
            if reps == 1:
                _emit_body(nc, tc, pools, aps, variant)
            else:
                with tc.For_i(0, reps, 1):
                    _emit_body(nc, tc, pools, aps, variant)

    dedupe_ldweights(nc)
    nc.compile()
    return nc


def make_in_maps(x, Wq, Wk, Wv):
    import ml_dtypes
    bf16 = ml_dtypes.bfloat16
    x = np.asarray(x, np.float32).reshape(B, S, D)
    Wqb = np.ascontiguousarray(Wq, np.float32).astype(bf16)
    Wkb = np.ascontiguousarray(Wk, np.float32).astype(bf16)
    Wvb = np.ascontiguousarray(Wv, np.float32).astype(bf16)
    ones = np.ones((P, 1), dtype=bf16)
    xT = [np.ascontiguousarray(x[b].T.astype(bf16)) for b in range(B)]

    # per-parity mask: at k-tile t the slot in its masking window is p = t//2
    # (tile ta = QTILES[r][p]); maskT[k_local, q_local] = NEG where
    # global key > global query
    masks = []
    kk = np.arange(P)[:, None]
    qq = np.arange(P)[None, :]
    for r in range(2):
        mA = np.zeros((P, NT * P), np.float32)
        for t in range(NT):
            ta = QTILES[r][t // 2]
            mA[:, t * P:(t + 1) * P] = \
                np.where(t * P + kk > ta * P + qq, NEG, 0.0)
        masks.append(mA)

    in_maps = []
    for c in range(NCORES):
        b, r = c // 2, c % 2
        cols = np.concatenate([np.arange(t * P, (t + 1) * P) for t in QTILES[r]])
        xqTb = np.ascontiguousarray(xT[b][:, cols])
        in_maps.append({
            "xT": xT[b], "xqT": xqTb, "Wq": Wqb, "Wk": Wkb, "Wv": Wvb,
            "maskA": masks[r], "ones": ones,
        })
    return in_maps


def assemble_output(results):
    out = np.empty((B, S, D), dtype=np.float32)
    for c in range(NCORES):
        b, r = c // 2, c % 2
        oc = np.asarray(results[c]["O"]).astype(np.float32).reshape(SLOTS, P, D)
        for sl in range(SLOTS):
            t = QTILES[r][sl]
            out[b, t * P:(t + 1) * P, :] = oc[sl]
    return out


_nc_cache = {}


def _get_program(reps: int = 1):
    if reps not in _nc_cache:
        _nc_cache[reps] = build_program(reps)
    return _nc_cache[reps]


def kernel(x, Wq, Wk, Wv):
    x = np.asarray(x, dtype=np.float32)
    Wq = np.asarray(Wq, dtype=np.float32)
    Wk = np.asarray(Wk, dtype=np.float32)
    Wv = np.asarray(Wv, dtype=np.float32)
    nc = _get_program(1)
    in_maps = make_in_maps(x, Wq, Wk, Wv)
    results = run_bass_kernel_spmd(nc, in_maps, list(range(NCORES))).results
    return assemble_output(results)


# revision 28
# speedup vs baseline: 1.0518x; 1.0518x over previous
"""Causal single-head attention (B=4, S=2048, D=1024, fp32) on 8 TRN2 NeuronCores.

Sharding: core c <-> (batch c//2, parity c%2). Each core owns 8 of the 16
128-row query tiles of its batch, chosen so total causal work is balanced
(parity 0: tiles {0,3,4,7,8,11,12,15}, parity 1: {1,2,5,6,9,10,13,14}; both
sum to 68 causal blocks). All compute is bf16 into fp32 PSUM (tolerance 2e-2
admits it; bf16 runs the PE at full rate and halves SBUF/DMA).

Attention is computed transposed: scoresT[k,q] = sum_e kt[e].T @ qt[e], so the
exp output P^T[k,q] lands in SBUF directly in the layout the PV matmul needs
as its stationary operand — no PE transposes, no PSUM->SBUF P copies, no
cross-engine round-trips on the PE critical path. Softmax denominators come
from a 1-column matmul against a ones vector reusing the already-loaded P^T
stationary. Causal masking is data-driven (SPMD-uniform program): per-parity
mask tensors cover the last two k-tiles of each query tile's range.

Self-contained: hardcodes shapes; reads nothing from disk.
"""
import sys

import numpy as np

try:
    from concourse import bass, bacc, tile
except ImportError:  # concourse ships with the container, not this file
    for _p in ("/opt/trn_rl_repo", "/root/.axon_site/_ro/trn_rl_repo"):
        if _p not in sys.path:
            sys.path.append(_p)
    from concourse import bass, bacc, tile
from concourse import mybir
from concourse.bass_utils import run_bass_kernel_spmd

dt = mybir.dt
AF = mybir.ActivationFunctionType

B, S, D = 4, 2048, 1024
P = 128
ND = D // P          # 8 d-tiles (contraction of projections)
NE = D // P          # 8 e-tiles (output feature tiles)
NT = S // P          # 16 s-tiles per batch
SLOTS = 8            # q-tiles per core
NG = 4               # query-tile groups of 2 per core
NCORES = 8
SCALE = 1.0 / float(np.sqrt(D))
NEG = -1.0e30

# Balanced query-tile assignment per parity (both sum to 68 causal blocks),
# listed in ascending causal-extent order so slot p's extent <= ES[p].
QTILES = [
    [0, 3, 4, 7, 8, 11, 12, 15],
    [1, 2, 5, 6, 9, 10, 13, 14],
]
# Uniform processed extent (k-tiles) per slot, identical for both parities.
ES = [2 * p + 2 for p in range(SLOTS)]


def _emit_body(nc, tc, pools, aps, variant="full"):
    do_proj = variant in ("full", "proj")
    do_attn = variant in ("full", "attn", "scores")
    (sb_x, sb_w, sb_qt, sb_kt, sb_vt, sb_p, sb_msk, sb_one,
     sb_rec, sb_obuf, po, ps_dn) = pools
    xT, xqT, Wq, Wk, Wv, maskA, ones, O = aps

    ones_sb = sb_one.tile([P, 1], dt.bfloat16, tag="ones", name="ones_sb")
    nc.sync.dma_start(ones_sb[:], ones[:])
    mA = sb_msk.tile([P, NT * P], dt.bfloat16, tag="mA", name="mA")
    nc.sync.dma_start(mA[:], maskA[:])

    # ---- input DMAs: x^T (all keys), xq^T (own query cols), weights (natural rows)
    xt = [sb_x.tile([P, S], dt.bfloat16, tag=f"xt{d}", name=f"xt{d}")
          for d in range(ND)]
    xqt = [sb_x.tile([P, SLOTS * P], dt.bfloat16, tag=f"xqt{d}", name=f"xqt{d}")
           for d in range(ND)]
    wk = [sb_w.tile([P, D], dt.bfloat16, tag=f"wk{d}", name=f"wk{d}") for d in range(ND)]
    wq = [sb_w.tile([P, D], dt.bfloat16, tag=f"wq{d}", name=f"wq{d}") for d in range(ND)]
    wv = [sb_w.tile([P, D], dt.bfloat16, tag=f"wv{d}", name=f"wv{d}") for d in range(ND)]
    for d in range(ND):
        nc.sync.dma_start(xt[d][:], xT[d * P:(d + 1) * P, :])
        nc.sync.dma_start(wk[d][:], Wk[d * P:(d + 1) * P, :])
    for d in range(ND):
        nc.sync.dma_start(xqt[d][:], xqT[d * P:(d + 1) * P, :])
        nc.sync.dma_start(wq[d][:], Wq[d * P:(d + 1) * P, :])
        nc.sync.dma_start(wv[d][:], Wv[d * P:(d + 1) * P, :])

    # ---- K^T projection: kt[e][:, k] = sum_d Wk[d,e]^T x^T[d,k]
    kt = [sb_kt.tile([P, S], dt.bfloat16, tag=f"kt{e}", name=f"kt{e}")
          for e in range(NE)]
    for e in range(NE) if do_proj else []:
        for h in range(2):
            kp = po.tile([P, 2 * 512], dt.float32, tag="o", name="kp")
            for d in range(ND):
                for c in range(2):
                    nc.tensor.matmul(kp[:, c * 512:(c + 1) * 512],
                                     wk[d][:, e * P:(e + 1) * P],
                                     xt[d][:, h * 1024 + c * 512:h * 1024 + (c + 1) * 512],
                                     start=(d == 0), stop=(d == ND - 1))
            nc.vector.tensor_copy(kt[e][:, h * 1024:h * 1024 + 512], kp[:, 0:512])
            nc.vector.tensor_copy(kt[e][:, h * 1024 + 512:(h + 1) * 1024], kp[:, 512:1024])

    # ---- Q^T projection (own query columns only)
    qt = [sb_qt.tile([P, SLOTS * P], dt.bfloat16, tag=f"qt{e}", name=f"qt{e}")
          for e in range(NE)]
    for e in range(NE) if do_proj else []:
        qp = po.tile([P, 2 * 512], dt.float32, tag="o", name="qp")
        for d in range(ND):
            for c in range(2):
                nc.tensor.matmul(qp[:, c * 512:(c + 1) * 512],
                                 wq[d][:, e * P:(e + 1) * P],
                                 xqt[d][:, c * 512:(c + 1) * 512],
                                 start=(d == 0), stop=(d == ND - 1))
        nc.vector.tensor_copy(qt[e][:, 0:512], qp[:, 0:512])
        nc.vector.tensor_copy(qt[e][:, 512:1024], qp[:, 512:1024])

    # ---- V projection (natural [k, e] layout): vt[t][:, e] = sum_d x^T[d,kt]^T Wv[d,e]
    vt = [sb_vt.tile([P, D], dt.bfloat16, tag=f"vt{t}", name=f"vt{t}")
          for t in range(NT)]
    for t in range(NT) if do_proj else []:
        vp = po.tile([P, 2 * 512], dt.float32, tag="o", name="vp")
        for d in range(ND):
            for c in range(2):
                nc.tensor.matmul(vp[:, c * 512:(c + 1) * 512],
                                 xt[d][:, t * P:(t + 1) * P],
                                 wv[d][:, c * 512:(c + 1) * 512],
                                 start=(d == 0), stop=(d == ND - 1))
        nc.vector.tensor_copy(vt[t][:, 0:512], vp[:, 0:512])
        nc.vector.tensor_copy(vt[t][:, 512:1024], vp[:, 512:1024])

    if not do_proj and do_attn:
        # experiment variant: load "projected" tensors directly (garbage values)
        for e in range(NE):
            nc.sync.dma_start(kt[e][:], xT[e * P:(e + 1) * P, :])
            nc.sync.dma_start(qt[e][:], xqT[e * P:(e + 1) * P, :])
        for t in range(NT):
            nc.sync.dma_start(vt[t][:], Wv[(t % ND) * P:(t % ND + 1) * P, :])

    # ---- S phase: scoresT for all own slots at once, ragged causal widths.
    # At k-tile t, slots t//2..7 are still active; slot t//2 is in its masking
    # window, so the mask always lands on columns 0:128 of the score tile.
    # exp output P^T goes to SBUF, aliased into the spent xt/xqt buffers.
    pts = [None] * NT
    if do_attn:
        # interleave wide/narrow k-tiles so the per-tile mask+exp round-trip is
        # always hidden behind a wide neighbor's matmul stream
        order = []
        for i in range(NT // 2):
            order += [i, NT - 1 - i]
        for t in order:
            off = (t // 2) * P
            wide = SLOTS * P - off
            s_t = po.tile([P, wide], dt.float32, tag="o", name="s_t")
            chunks = [(0, 512), (512, wide - 512)] if wide > 512 else [(0, wide)]
            for e in range(NE):
                for c0, cw in chunks:
                    nc.tensor.matmul(s_t[:, c0:c0 + cw],
                                     kt[e][:, t * P:(t + 1) * P],
                                     qt[e][:, off + c0:off + c0 + cw],
                                     start=(e == 0), stop=(e == NE - 1))
            nc.vector.tensor_add(s_t[:, 0:P], s_t[:, 0:P],
                                 mA[:, t * P:(t + 1) * P])
            pt = sb_p.tile([P, wide], dt.bfloat16, tag=f"pt{t}", name=f"pt{t}")
            nc.scalar.activation(pt[:], s_t[:], AF.Exp, scale=SCALE)
            pts[t] = pt

    # ---- PV passes: slot pairs (2pp, 2pp+1), accumulators live in PSUM
    for pp in range(NG) if (do_attn and variant != "scores") else []:
        pa, pb = 2 * pp, 2 * pp + 1
        ea, eb = ES[pa], ES[pb]
        oa = po.tile([P, D], dt.float32, tag="o", name="oa")
        ob = po.tile([P, D], dt.float32, tag="o", name="ob")
        # separate tiles: a start=True matmul clears has_written for its whole
        # PSUM bank, so the two running denominators must not share a bank
        dna = ps_dn.tile([P, 1], dt.float32, tag="dn", name="dna")
        dnb = ps_dn.tile([P, 1], dt.float32, tag="dn", name="dnb")
        for t in range(eb):
            off = (t // 2) * P
            ca = pa * P - off
            cb = pb * P - off
            pt = pts[t]
            if t < ea:
                for c in range(2):
                    nc.tensor.matmul(oa[:, c * 512:(c + 1) * 512], pt[:, ca:ca + P],
                                     vt[t][:, c * 512:(c + 1) * 512],
                                     start=(t == 0), stop=(t == ea - 1))
                nc.tensor.matmul(dna[:], pt[:, ca:ca + P], ones_sb[:],
                                 start=(t == 0), stop=(t == ea - 1))
            for c in range(2):
                nc.tensor.matmul(ob[:, c * 512:(c + 1) * 512], pt[:, cb:cb + P],
                                 vt[t][:, c * 512:(c + 1) * 512],
                                 start=(t == 0), stop=(t == eb - 1))
            nc.tensor.matmul(dnb[:], pt[:, cb:cb + P], ones_sb[:],
                             start=(t == 0), stop=(t == eb - 1))

        rec = sb_rec.tile([P, 2], dt.float32, tag="rec", name="rec")
        nc.vector.reciprocal(rec[:, 0:1], dna[:])
        nc.vector.reciprocal(rec[:, 1:2], dnb[:])
        obufa = sb_obuf.tile([P, D], dt.bfloat16, tag="obuf", name="obufa")
        obufb = sb_obuf.tile([P, D], dt.bfloat16, tag="obuf", name="obufb")
        for c in range(2):
            nc.scalar.mul(obufa[:, c * 512:(c + 1) * 512],
                          oa[:, c * 512:(c + 1) * 512], rec[:, 0:1])
            nc.scalar.mul(obufb[:, c * 512:(c + 1) * 512],
                          ob[:, c * 512:(c + 1) * 512], rec[:, 1:2])
        nc.sync.dma_start(O[pa * P:(pa + 1) * P, :], obufa[:])
        nc.sync.dma_start(O[pb * P:(pb + 1) * P, :], obufb[:])


def dedupe_ldweights(nc):
    """Drop InstLdweights whose stationary operand is already loaded.

    The PE keeps the stationary operand in the array across matmuls; Tile emits
    an Ldweights before every matmul regardless, and those loads serialize with
    the matmul stream (~92ns each). Scan each block in final (post-scheduling)
    order and delete loads identical to the previous one, tracking only PE
    instructions; reset on anything else PE-adjacent (drains, calls, branches).
    Only loads with no semaphore waits/updates are removed.
    """
    dropped = 0
    for f in nc.m.functions:
        for blk in f.blocks:
            keep = []
            last = None
            for inst in blk.instructions:
                if isinstance(inst, mybir.InstLdweights):
                    w = inst.ins[0]
                    key = (w.memref, w.offset, str(w.ap), w.dtype,
                           inst.is_transpose, inst.perf_mode)
                    si = inst.sync_info
                    clean = si is None or (not len(si.on_wait) and not len(si.on_update))
                    if key == last and clean:
                        dropped += 1
                        continue
                    last = key
                elif isinstance(inst, mybir.InstMatmult):
                    pass  # matmuls don't change the loaded stationary
                elif isinstance(inst, (mybir.InstTensorCopy, mybir.InstTensorTensor,
                                       mybir.InstActivation, mybir.InstReciprocal,
                                       mybir.InstDMACopy, mybir.InstMemset,
                                       mybir.InstEventSemaphore)):
                    pass  # other engines: PE weight state unaffected
                else:
                    last = None  # drains/calls/branches: be conservative
                keep.append(inst)
            if len(keep) != len(blk.instructions):
                del blk.instructions[:]
                for inst in keep:
                    blk.instructions.append(inst)
    return dropped


def build_program(reps: int = 1, variant: str = "full"):
    nc = bacc.Bacc("TRN2", target_bir_lowering=False, debug=False, num_devices=NCORES)

    xT_t = nc.dram_tensor("xT", [D, S], dt.bfloat16, kind="ExternalInput")
    xqT_t = nc.dram_tensor("xqT", [D, SLOTS * P], dt.bfloat16, kind="ExternalInput")
    Wq_t = nc.dram_tensor("Wq", [D, D], dt.bfloat16, kind="ExternalInput")
    Wk_t = nc.dram_tensor("Wk", [D, D], dt.bfloat16, kind="ExternalInput")
    Wv_t = nc.dram_tensor("Wv", [D, D], dt.bfloat16, kind="ExternalInput")
    maskA_t = nc.dram_tensor("maskA", [P, NT * P], dt.bfloat16, kind="ExternalInput")
    ones_t = nc.dram_tensor("ones", [P, 1], dt.bfloat16, kind="ExternalInput")
    O_t = nc.dram_tensor("O", [SLOTS * P, D], dt.bfloat16, kind="ExternalOutput")

    aps = (xT_t.ap(), xqT_t.ap(), Wq_t.ap(), Wk_t.ap(), Wv_t.ap(),
           maskA_t.ap(), ones_t.ap(), O_t.ap())

    with tile.TileContext(nc) as tc:
        with (
            tc.tile_pool(name="x", bufs=1) as sb_x,
            tc.tile_pool(name="w", bufs=1) as sb_w,
            tc.tile_pool(name="qt", bufs=1) as sb_qt,
            tc.tile_pool(name="kt", bufs=1) as sb_kt,
            tc.tile_pool(name="vt", bufs=1) as sb_vt,
            tc.tile_pool(name="p", bufs=1) as sb_p,
            tc.tile_pool(name="msk", bufs=1) as sb_msk,
            tc.tile_pool(name="one", bufs=1) as sb_one,
            tc.tile_pool(name="rec", bufs=2) as sb_rec,
            tc.tile_pool(name="obuf", bufs=3) as sb_obuf,
            tc.tile_pool(name="po", bufs=3, space=bass.MemorySpace.PSUM) as po,
            tc.tile_pool(name="ps_dn", bufs=2, space=bass.MemorySpace.PSUM) as ps_dn,
        ):
            pools = (sb_x, sb_w, sb_qt, sb_kt, sb_vt, sb_p, sb_msk, sb_one,
                     sb_rec, sb_obuf, po, ps_dn)
            if reps == 1:
                _emit_body(nc, tc, pools, aps, variant)
            else:
                with tc.For_i(0, reps, 1):
                    _emit_body(nc, tc, pools, aps, variant)

    dedupe_ldweights(nc)
    nc.compile()
    return nc


def make_in_maps(x, Wq, Wk, Wv):
    import ml_dtypes
    bf16 = ml_dtypes.bfloat16
    x = np.asarray(x, np.float32).reshape(B, S, D)
    Wqb = np.ascontiguousarray(Wq, np.float32).astype(bf16)
    Wkb = np.ascontiguousarray(Wk, np.float32).astype(bf16)
    Wvb = np.ascontiguousarray(Wv, np.float32).astype(bf16)
    ones = np.ones((P, 1), dtype=bf16)
    xT = [np.ascontiguousarray(x[b].T.astype(bf16)) for b in range(B)]

    # per-parity mask: at k-tile t the slot in its masking window is p = t//2
    # (tile ta = QTILES[r][p]); maskT[k_local, q_local] = NEG where
    # global key > global query
    masks = []
    kk = np.arange(P)[:, None]
    qq = np.arange(P)[None, :]
    for r in range(2):
        mA = np.zeros((P, NT * P), np.float32)
        for t in range(NT):
            ta = QTILES[r][t // 2]
            mA[:, t * P:(t + 1) * P] = \
                np.where(t * P + kk > ta * P + qq, NEG, 0.0)
        masks.append(mA.astype(bf16))

    in_maps = []
    for c in range(NCORES):
        b, r = c // 2, c % 2
        cols = np.concatenate([np.arange(t * P, (t + 1) * P) for t in QTILES[r]])
        xqTb = np.ascontiguousarray(xT[b][:, cols])
        in_maps.append({
            "xT": xT[b], "xqT": xqTb, "Wq": Wqb, "Wk": Wkb, "Wv": Wvb,
            "maskA": masks[r], "ones": ones,
        })
    return in_maps


def assemble_output(results):
    out = np.empty((B, S, D), dtype=np.float32)
    for c in range(NCORES):
        b, r = c // 2, c % 2
        oc = np.asarray(results[c]["O"]).astype(np.float32).reshape(SLOTS, P, D)
        for sl in range(SLOTS):
            t = QTILES[r][sl]
            out[b, t * P:(t + 1) * P, :] = oc[sl]
    return out


_nc_cache = {}


def _get_program(reps: int = 1):
    if reps not in _nc_cache:
        _nc_cache[reps] = build_program(reps)
    return _nc_cache[reps]


def kernel(x, Wq, Wk, Wv):
    x = np.asarray(x, dtype=np.float32)
    Wq = np.asarray(Wq, dtype=np.float32)
    Wk = np.asarray(Wk, dtype=np.float32)
    Wv = np.asarray(Wv, dtype=np.float32)
    nc = _get_program(1)
    in_maps = make_in_maps(x, Wq, Wk, Wv)
    results = run_bass_kernel_spmd(nc, in_maps, list(range(NCORES))).results
    return assemble_output(results)
